# revision 1
# baseline (speedup 1.0000x reference)
"""AdaConv2d on 8 TRN2 NeuronCores.

Per-sample adaptive 3x3 conv (stride 1, pad 1): each sample b uses
kernel_base * kernel_mask[demog_label[b]].

Strategy: data-parallel over batch (8 samples/core). Host gathers the
per-sample mask (by label) and pre-pads x; device computes the per-sample
masked kernel (9 tensor_scalar_mul) and the conv as 9 shifted bf16 matmuls
(K=IC=128, M=128 oc-block, N=448 pixels) accumulating in PSUM.
"""

import numpy as np
from ml_dtypes import bfloat16

NCORES = 8
BS = 8            # samples per core
IC, OC, KS = 128, 256, 3
H = W = 56
HP = WP = 58      # padded
NPIX = H * W      # 3136
RROWS = 8         # output rows per matmul tile
RT = H // RROWS   # 7 row-tiles
NTAP = KS * KS    # 9
NFREE = RROWS * W # 448

_cached_nc = None


def _build():
    import concourse.mybir as mybir
    import concourse.bacc as bacc
    import concourse.tile as tile

    nc = bacc.Bacc("TRN2", target_bir_lowering=False, debug=False)
    bf = mybir.dt.bfloat16
    f32 = mybir.dt.float32

    x_ext = nc.declare_dram_parameter("x", [BS, IC, HP, WP], bf, isOutput=False)
    kb_ext = nc.declare_dram_parameter("kb", [IC, NTAP, OC], bf, isOutput=False)
    w0_ext = nc.declare_dram_parameter("w0", [IC, NTAP, OC], bf, isOutput=False)
    mk_ext = nc.declare_dram_parameter("mk", [IC, BS * NTAP], f32, isOutput=False)
    out_ext = nc.declare_dram_parameter("out", [BS, 2, 128, NPIX], f32, isOutput=True)

    with tile.TileContext(nc) as tc:
        with (
            tc.tile_pool(name="const", bufs=1) as cpool,
            tc.tile_pool(name="xin", bufs=3) as xpool,
            tc.tile_pool(name="wgt", bufs=2) as wpool,
            tc.tile_pool(name="ostage", bufs=3) as opool,
            tc.tile_pool(name="psum", bufs=7, space="PSUM") as pspool,
        ):
            # PE warmup: ~4us of dummy matmuls with no input deps so the HAM
            # clock-gate reaches 8/8 before the first real matmul.
            wub = cpool.tile([IC, 448], bf)
            nc.vector.memset(wub[:], 0.0)
            wps = pspool.tile([128, 448], f32, name="wups", tag="wups", bufs=1)
            for _ in range(12):
                nc.tensor.matmul(wps[:], wub[:, :128], wub[:], start=True, stop=True)

            # Issue order tracks the first-matmul critical path: the tiny mask
            # and first kernel tap first (they feed w[tap0]), then x[0] split
            # so its first row-tiles arrive early, then the rest.
            # sample 0's weights arrive pre-multiplied from the host: its
            # matmul stream depends only on these DMAs, not the DVE chain.
            w0 = cpool.tile([IC, NTAP, OC], bf)
            nc.sync.dma_start(w0[:, 0:3, :], w0_ext[:, 0:3, :])
            xp0 = xpool.tile([IC, HP, WP], bf, name="xp0", tag="xp")
            nc.sync.dma_start(xp0[:, :18, :], x_ext[0, :, :18, :])
            nc.sync.dma_start(w0[:, 3:9, :], w0_ext[:, 3:9, :])
            nc.sync.dma_start(xp0[:, 18:, :], x_ext[0, :, 18:, :])
            mk = cpool.tile([IC, BS * NTAP], f32)
            nc.sync.dma_start(mk[:], mk_ext[:])
            kb = cpool.tile([IC, NTAP, OC], bf)
            nc.sync.dma_start(kb[:], kb_ext[:])

            for s in range(BS):
                if s == 0:
                    xp = xp0
                else:
                    xp = xpool.tile([IC, HP, WP], bf, name=f"xp{s}", tag="xp")
                    nc.sync.dma_start(xp[:], x_ext[s])

                # sample 0 skips the shifted copy: its DVE queue must run the
                # 9 weight-prep ops unobstructed or the stream start stalls
                # ~1.9us; 54 odd-aligned matmuls only cost ~0.4us.
                if s == 0:
                    x2 = None
                else:
                    x2 = xpool.tile([IC, HP, WP], bf, name=f"x2_{s}", tag="xp2")
                    nc.vector.tensor_copy(x2[:, :, :57], xp[:, :, 1:58])

                if s == 0:
                    w = w0
                else:
                    w = wpool.tile([IC, NTAP, OC], bf, name=f"w{s}", tag="w")
                    for t in range(NTAP):
                        nc.vector.tensor_scalar_mul(
                            w[:, t, :], kb[:, t, :],
                            mk[:, s * NTAP + t : s * NTAP + t + 1],
                        )

                for ocb in range(2):
                    ost = opool.tile([128, NPIX], f32, name=f"ost{s}_{ocb}", tag="ost")
                    for rt in range(RT):
                        ps = pspool.tile([128, NFREE], f32, name=f"ps{s}_{ocb}_{rt}", tag="ps")
                        for t in range(NTAP):
                            kh, kw = divmod(t, KS)
                            if kw == 1 and x2 is not None:
                                # odd-element rhs starts cost ~7ns/matmul on
                                # the PE: read the center taps from the
                                # 1-shifted copy at even alignment instead.
                                rhs = x2[:, rt * RROWS + kh : rt * RROWS + kh + RROWS, 0:W]
                            else:
                                rhs = xp[:, rt * RROWS + kh : rt * RROWS + kh + RROWS, kw : kw + W]
                            nc.tensor.matmul(
                                ps[:],
                                w[:, t, ocb * 128 : (ocb + 1) * 128],
                                rhs,
                                start=(t == 0),
                                stop=(t == NTAP - 1),
                            )
                        nc.any.tensor_copy(ost[:, rt * NFREE : (rt + 1) * NFREE], ps[:])
                        # chunked output DMA: each row-tile ships as soon as its
                        # drain lands, so the kernel tail is one small DMA. The
                        # last sample goes on the ScalarE HWDGE ring so its
                        # chunks don't queue behind the Sync ring's backlog.
                        dma_eng = nc.scalar if s == BS - 1 else nc.sync
                        dma_eng.dma_start(
                            out_ext[s, ocb, :, rt * NFREE : (rt + 1) * NFREE],
                            ost[:, rt * NFREE : (rt + 1) * NFREE],
                        )
    nc.compile()
    return nc


def run(inputs, trace=False, **kw):
    from concourse.bass_utils import run_bass_kernel_spmd

    global _cached_nc
    if _cached_nc is None:
        _cached_nc = _build()
    nc = _cached_nc

    x = np.asarray(inputs["x"])
    demog_label = np.asarray(inputs["demog_label"])
    kernel_base = np.asarray(inputs["kernel_base"])
    kernel_mask = np.asarray(inputs["kernel_mask"])
    B = x.shape[0]

    xpad = np.zeros((B, IC, HP, WP), dtype=bfloat16)
    xpad[:, :, 1 : H + 1, 1 : W + 1] = x.astype(bfloat16)

    # [OC, IC, 3, 3] -> [IC, 9, OC]
    kb_t = np.ascontiguousarray(
        kernel_base.transpose(1, 2, 3, 0).reshape(IC, NTAP, OC)
    ).astype(bfloat16)
    # gather per-sample mask by label: [B, IC, 9]
    mask_t = kernel_mask[demog_label].reshape(B, IC, NTAP).astype(np.float32)

    in_maps = []
    for c in range(NCORES):
        sl = slice(c * BS, (c + 1) * BS)
        mk_c = np.ascontiguousarray(
            mask_t[sl].transpose(1, 0, 2).reshape(IC, BS * NTAP)
        )
        w0_c = np.ascontiguousarray(
            kb_t.astype(np.float32)
            * mask_t[c * BS][:, :, None].astype(np.float32)
        ).astype(bfloat16)
        in_maps.append({
            "x": np.ascontiguousarray(xpad[sl]),
            "kb": kb_t,
            "mk": mk_c,
            "w0": w0_c,
        })

    last_exc = None
    for _attempt in range(3):
        try:
            res = run_bass_kernel_spmd(nc, in_maps, core_ids=list(range(NCORES)),
                                       trace=trace, **kw)
            # force materialization here so transient device faults are
            # caught by this retry loop rather than surfacing later
            outs = [np.asarray(r["out"]).reshape(BS, OC, H, W)
                    for r in res.results]
            full = np.concatenate(outs, axis=0).astype(np.float32)
            return full, res
        except Exception as e:  # transient NRT/device faults: retry
            last_exc = e
            import time
            time.sleep(10)
    raise last_exc


def kernel(**inputs):
    out, _ = run(inputs, trace=False)
    return out



# revision 2
# speedup vs baseline: 1.0331x; 1.0331x over previous
"""AdaConv2d on 8 TRN2 NeuronCores — Winograd F(4,3) along W.

Per-sample adaptive 3x3 conv (stride 1, pad 1): sample b uses
kernel_base * kernel_mask[demog_label[b]].

Data-parallel over batch (8 samples/core). The host computes the
Winograd input transform V = B^T d (per 4-wide output tile along W) and
the per-sample transformed weights W[i,kh] = sum_kw G[i,kw] k[kh,kw]
(the label gather is also host-side). The device computes, per
(sample, oc-block), 6 Winograd planes M[i] = sum_kh W[i,kh] @ V[i,kh]
as PSUM-accumulated matmuls (2.0x fewer PE columns than direct conv),
drains them to SBUF bf16 on ScalarE, and applies the inverse transform
A^T M on the DVE (bf16 2x tensor_tensor + 4x tensor_scalar). Output is
written phase-split [q][h][j] (w = 4j+q) in bf16; the host
de-interleaves and casts to f32.
"""

import numpy as np
from ml_dtypes import bfloat16

NCORES = 8
BS = 8              # samples per core
IC, OC, KS = 128, 256, 3
H = W = 56
HP = 58             # h-padded rows
NT = 14             # Winograd tiles along W (4 outputs each)
A = 6               # Winograd input tile size
NKH = 3
RC = 28             # output rows per psum chunk
NC = H // RC        # 2 chunks
FDC = RC * NT       # 392 psum columns per chunk
FDP = H * NT        # 784 columns per full plane

_cached_nc = None

# F(4,3) transform matrices (Lavin), f64 for host-side precision.
BT4 = np.array([
    [4, 0, -5, 0, 1, 0],
    [0, -4, -4, 1, 1, 0],
    [0, 4, -4, -1, 1, 0],
    [0, -2, -1, 2, 1, 0],
    [0, 2, -1, -2, 1, 0],
    [0, 4, 0, -5, 0, 1]], np.float64)
G4 = np.array([
    [1 / 4, 0, 0],
    [-1 / 6, -1 / 6, -1 / 6],
    [-1 / 6, 1 / 6, -1 / 6],
    [1 / 24, 1 / 12, 1 / 6],
    [1 / 24, -1 / 12, 1 / 6],
    [0, 0, 1]], np.float64)


def _build():
    import concourse.mybir as mybir
    import concourse.bacc as bacc
    import concourse.tile as tile

    nc = bacc.Bacc("TRN2", target_bir_lowering=False, debug=False)
    bf = mybir.dt.bfloat16
    f32 = mybir.dt.float32

    v_ext = nc.declare_dram_parameter("v", [BS, IC, A, HP, NT], bf, isOutput=False)
    w_ext = nc.declare_dram_parameter("w", [BS, IC, A * NKH, OC], bf, isOutput=False)
    out_ext = nc.declare_dram_parameter("out", [BS, 2, 128, 4, FDP], bf, isOutput=True)

    with tile.TileContext(nc) as tc:
        with (
            tc.tile_pool(name="const", bufs=1) as cpool,
            tc.tile_pool(name="vin", bufs=3) as vpool,
            tc.tile_pool(name="wgt", bufs=3) as wpool,
            tc.tile_pool(name="mst", bufs=2) as mpool,
            tc.tile_pool(name="itm", bufs=16) as ipool,
            tc.tile_pool(name="yst", bufs=3) as ypool,
            tc.tile_pool(name="psum", bufs=7, space="PSUM") as pspool,
        ):
            # PE warmup: dummy matmuls with no input deps so the HAM
            # clock-gate reaches 8/8 before the first real matmul.
            wub = cpool.tile([IC, FDC], bf)
            nc.vector.memset(wub[:], 0.0)
            wps = pspool.tile([128, FDC], f32, name="wups", tag="wups", bufs=1)
            for _ in range(12):
                nc.tensor.matmul(wps[:], wub[:, :128], wub[:], start=True, stop=True)

            vt = {}
            wt = {}
            for s in range(BS):
                vt[s] = vpool.tile([IC, A, HP, NT], bf, name=f"v{s}", tag="v")
                nc.sync.dma_start(vt[s][:], v_ext[s])
                wt[s] = wpool.tile([IC, A * NKH, OC], bf, name=f"w{s}", tag="w")
                nc.sync.dma_start(wt[s][:], w_ext[s])

            for s in range(BS):
                for ocb in range(2):
                    ms = mpool.tile([128, A, FDP], bf, name=f"m{s}_{ocb}", tag="m")
                    # 6 Winograd planes x 2 chunks of matmuls; kh accumulates
                    # in PSUM, chunks interleave so each lhsT load serves 2
                    # matmuls.
                    ps = {}
                    for i in range(A):
                        for c in range(NC):
                            ps[c] = pspool.tile(
                                [128, FDC], f32, name=f"ps{s}_{ocb}_{i}_{c}", tag="ps"
                            )
                        for kh in range(NKH):
                            lhs = wt[s][:, i * NKH + kh, ocb * 128:(ocb + 1) * 128]
                            for c in range(NC):
                                nc.tensor.matmul(
                                    ps[c][:],
                                    lhs,
                                    vt[s][:, i, RC * c + kh: RC * c + kh + RC, :],
                                    start=(kh == 0),
                                    stop=(kh == NKH - 1),
                                )
                        for c in range(NC):
                            nc.scalar.copy(
                                ms[:, i, c * FDC:(c + 1) * FDC], ps[c][:]
                            )

                    # inverse transform on DVE: all-bf16 SBUF ops (2x TT, 4x TS)
                    def it(nm):
                        return ipool.tile([128, FDP], bf, name=f"{nm}{s}_{ocb}", tag="i")

                    y = ypool.tile([128, 4, FDP], bf, name=f"y{s}_{ocb}", tag="y")
                    ta, tb, tc_, td = it("a"), it("b"), it("c"), it("d")
                    t0, d2, c4, d8, tu = it("t0"), it("d2"), it("c4"), it("d8"), it("u")
                    nc.vector.tensor_add(ta[:], ms[:, 1, :], ms[:, 2, :])
                    nc.vector.tensor_sub(tb[:], ms[:, 1, :], ms[:, 2, :])
                    nc.vector.tensor_add(tc_[:], ms[:, 3, :], ms[:, 4, :])
                    nc.vector.tensor_sub(td[:], ms[:, 3, :], ms[:, 4, :])
                    nc.vector.tensor_add(t0[:], ta[:], tc_[:])
                    nc.vector.tensor_add(y[:, 0, :], t0[:], ms[:, 0, :])
                    nc.vector.tensor_scalar_mul(d2[:], td[:], 2.0)
                    nc.vector.tensor_add(y[:, 1, :], tb[:], d2[:])
                    nc.vector.tensor_scalar_mul(c4[:], tc_[:], 4.0)
                    nc.vector.tensor_add(y[:, 2, :], ta[:], c4[:])
                    nc.vector.tensor_scalar_mul(d8[:], td[:], 8.0)
                    nc.vector.tensor_add(tu[:], tb[:], d8[:])
                    nc.vector.tensor_add(y[:, 3, :], tu[:], ms[:, 5, :])

                    nc.scalar.dma_start(out_ext[s, ocb], y[:])
    nc.compile()
    return nc


def _host_prep(x, demog_label, kernel_base, kernel_mask):
    B = x.shape[0]
    # pad h and w by 1 (h to 58, w to 58)
    xpad = np.zeros((B, IC, HP, HP), np.float32)
    xpad[:, :, 1:H + 1, 1:W + 1] = x
    # input transform: V[b, ic, i, h, j] = sum_t BT4[i,t] xpad[b, ic, h, 4j+t]
    dwin = np.lib.stride_tricks.sliding_window_view(xpad, A, axis=3)[:, :, :, ::4, :]
    V = np.einsum("it,bchjt->bcihj", BT4.astype(np.float32), dwin,
                  optimize=True).astype(bfloat16)
    # weight transform: W[b, ic, i*3+kh, oc] = sum_kw G4[i,kw] kb[oc,ic,kh,kw] m[b,ic,kh,kw]
    mg = kernel_mask[demog_label]                        # [B, IC, 3, 3]
    km = np.einsum("ochw,bchw->bochw", kernel_base, mg, optimize=True)
    Wt = np.einsum("iw,bochw->bciho", G4.astype(np.float32),
                   km, optimize=True)                    # [B, IC, A, KH, OC]
    Wt = np.ascontiguousarray(Wt.reshape(B, IC, A * NKH, OC)).astype(bfloat16)
    return V, Wt


def _host_post(buf):
    # buf: [B, 2, 128, 4, 784] bf16 -> [B, 256, 56, 56] f32
    B = buf.shape[0]
    r = buf.reshape(B, 2, 128, 4, H, NT).astype(np.float32)
    r = r.transpose(0, 1, 2, 4, 5, 3)                    # [B, 2, 128, 56, 14, 4]
    return np.ascontiguousarray(r.reshape(B, OC, H, W))


def run(inputs, trace=False, **kw):
    from concourse.bass_utils import run_bass_kernel_spmd

    global _cached_nc
    if _cached_nc is None:
        _cached_nc = _build()
    nc = _cached_nc

    x = np.asarray(inputs["x"], dtype=np.float32)
    demog_label = np.asarray(inputs["demog_label"])
    kernel_base = np.asarray(inputs["kernel_base"], dtype=np.float32)
    kernel_mask = np.asarray(inputs["kernel_mask"], dtype=np.float32)

    V, Wt = _host_prep(x, demog_label, kernel_base, kernel_mask)

    in_maps = []
    for c in range(NCORES):
        sl = slice(c * BS, (c + 1) * BS)
        in_maps.append({
            "v": np.ascontiguousarray(V[sl]),
            "w": np.ascontiguousarray(Wt[sl]),
        })

    last_exc = None
    for _attempt in range(3):
        try:
            res = run_bass_kernel_spmd(nc, in_maps, core_ids=list(range(NCORES)),
                                       trace=trace, **kw)
            outs = [_host_post(np.asarray(r["out"])) for r in res.results]
            full = np.concatenate(outs, axis=0)
            return full, res
        except Exception as e:  # transient NRT/device faults: retry
            last_exc = e
            import time
            time.sleep(10)
    raise last_exc


def kernel(**inputs):
    out, _ = run(inputs, trace=False)
    return out


# revision 3
# speedup vs baseline: 1.0594x; 1.0254x over previous
"""AdaConv2d on 8 TRN2 NeuronCores — Winograd F(4,3) along W.

Per-sample adaptive 3x3 conv (stride 1, pad 1): sample b uses
kernel_base * kernel_mask[demog_label[b]].

Data-parallel over batch (8 samples/core). The host computes the
Winograd input transform V = B^T d (per 4-wide output tile along W) and
the per-sample transformed weights W[i,kh] = sum_kw G[i,kw] k[kh,kw]
(the label gather is also host-side). The device computes, per
(sample, oc-block), 6 Winograd planes M[i] = sum_kh W[i,kh] @ V[i,kh]
as PSUM-accumulated matmuls (2.0x fewer PE columns than direct conv),
drains each 3-plane PSUM group with a single strided ScalarE ACTIVATE,
and applies the inverse transform A^T M on the DVE as fused plane-pair
bf16 ops (2x tensor_tensor / 4x tensor_scalar). Output goes out
phase-split [q][h][j] (w = 4j+q) in bf16; the host de-interleaves and
casts to f32.
"""

import numpy as np
from ml_dtypes import bfloat16

NCORES = 8
BS = 8              # samples per core
IC, OC, KS = 128, 256, 3
H = W = 56
HP = 58             # h-padded rows
NT = 14             # Winograd tiles along W (4 outputs each)
A = 6               # Winograd input tile size
NKH = 3
RC = 28             # output rows per psum chunk
NC = H // RC        # 2 chunks
FDC = RC * NT       # 392 psum columns per chunk
FDP = H * NT        # 784 columns per full plane
PB = 512            # psum bank stride (f32 elems)

_cached_nc = None

# F(4,3) transform matrices (Lavin), f64 for host-side precision.
BT4 = np.array([
    [4, 0, -5, 0, 1, 0],
    [0, -4, -4, 1, 1, 0],
    [0, 4, -4, -1, 1, 0],
    [0, -2, -1, 2, 1, 0],
    [0, 2, -1, -2, 1, 0],
    [0, 4, 0, -5, 0, 1]], np.float64)
G4 = np.array([
    [1 / 4, 0, 0],
    [-1 / 6, -1 / 6, -1 / 6],
    [-1 / 6, 1 / 6, -1 / 6],
    [1 / 24, 1 / 12, 1 / 6],
    [1 / 24, -1 / 12, 1 / 6],
    [0, 0, 1]], np.float64)


def _build():
    import concourse.mybir as mybir
    import concourse.bacc as bacc
    import concourse.tile as tile

    nc = bacc.Bacc("TRN2", target_bir_lowering=False, debug=False)
    bf = mybir.dt.bfloat16
    f32 = mybir.dt.float32

    v_ext = nc.declare_dram_parameter("v", [BS, IC, A, HP, NT], bf, isOutput=False)
    w_ext = nc.declare_dram_parameter("w", [BS, IC, A * NKH, OC], bf, isOutput=False)
    out_ext = nc.declare_dram_parameter("out", [BS, 2, 128, 4, FDP], bf, isOutput=True)

    # out-DMA rings: alternate sync/gpsimd (ScalarE keeps only drains —
    # a doorbell waiting on combines would head-of-line-block later drains
    # in ScalarE's strict-FIFO queue). The last pair goes on sync (HWDGE,
    # ~0.6us first-byte vs ~2us SWDGE) to keep the kernel tail short.
    def out_dma(nc, idx, dst, src, last=False):
        eng = nc.sync if last else [nc.sync, nc.gpsimd][idx % 2]
        eng.dma_start(dst, src)

    with tile.TileContext(nc) as tc:
        with (
            tc.tile_pool(name="const", bufs=1) as cpool,
            tc.tile_pool(name="vin", bufs=3) as vpool,
            tc.tile_pool(name="wgt", bufs=3) as wpool,
            tc.tile_pool(name="mst", bufs=2) as mpool,
            tc.tile_pool(name="itm", bufs=2) as ipool,
            tc.tile_pool(name="yst", bufs=3) as ypool,
            tc.tile_pool(name="psum", bufs=2, space="PSUM") as pspool,
        ):
            # PE warmup: dummy matmuls with no input deps so the HAM
            # clock-gate reaches 8/8 before the first real matmul.
            wub = cpool.tile([IC, FDC], bf)
            nc.vector.memset(wub[:], 0.0)
            wps = pspool.tile([128, FDC], f32, name="wups", tag="wups", bufs=1)
            for _ in range(8):
                nc.tensor.matmul(wps[:], wub[:, :128], wub[:], start=True, stop=True)

            # sample 0's tiles arrive as two halves so the first matmuls
            # (planes i=0..2) can start as early as possible. Samples 1-2
            # prefetch up-front (pool depth 3); s+3 is issued at the end of
            # sample s so no DMA waits block the queues mid-kernel.
            vt = {}
            wt = {}

            def fetch(s, split=False):
                vt[s] = vpool.tile([IC, A, HP, NT], bf, name=f"v{s}", tag="v")
                wt[s] = wpool.tile([IC, A * NKH, OC], bf, name=f"w{s}", tag="w")
                if split:
                    nc.sync.dma_start(vt[s][:, 0:3], v_ext[s, :, 0:3])
                    nc.gpsimd.dma_start(wt[s][:, 0:9, :], w_ext[s, :, 0:9, :])
                    nc.sync.dma_start(vt[s][:, 3:6], v_ext[s, :, 3:6])
                    nc.gpsimd.dma_start(wt[s][:, 9:18, :], w_ext[s, :, 9:18, :])
                else:
                    nc.sync.dma_start(vt[s][:], v_ext[s])
                    nc.gpsimd.dma_start(wt[s][:], w_ext[s])

            fetch(0, split=True)
            fetch(1)
            fetch(2)

            ndma = 0
            for s in range(BS):
                for ocb in range(2):
                    ms = mpool.tile([128, A, FDP], bf, name=f"m{s}_{ocb}", tag="m")
                    for c in range(NC):
                        for half in range(2):
                            pst = pspool.tile(
                                [128, NKH, PB], f32,
                                name=f"ps{s}_{ocb}_{c}_{half}", tag="ps",
                            )
                            for i3 in range(NKH):
                                i = half * NKH + i3
                                for kh in range(NKH):
                                    nc.tensor.matmul(
                                        pst[:, i3, 0:FDC],
                                        wt[s][:, i * NKH + kh,
                                              ocb * 128:(ocb + 1) * 128],
                                        vt[s][:, i, RC * c + kh: RC * c + kh + RC, :],
                                        start=(kh == 0),
                                        stop=(kh == NKH - 1),
                                    )
                            # one strided drain for the whole 3-bank group
                            nc.scalar.copy(
                                ms[:, half * NKH:(half + 1) * NKH,
                                   c * FDC:(c + 1) * FDC],
                                pst[:, :, 0:FDC],
                            )

                    # inverse transform on DVE: fused plane-pair bf16 ops.
                    # W5 layout: [b, a, d, c, d8]; R: [d2, c4]; TU: [t0, u]
                    y = ypool.tile([128, 4, FDP], bf, name=f"y{s}_{ocb}", tag="y")
                    w5 = ipool.tile([128, 5, FDP], bf, name=f"w5_{s}_{ocb}", tag="w5")
                    rr = ipool.tile([128, 2, FDP], bf, name=f"r_{s}_{ocb}", tag="r")
                    tu = ipool.tile([128, 2, FDP], bf, name=f"tu_{s}_{ocb}", tag="tu")
                    # the last pair runs per-chunk so the kernel tail is short
                    parts = [slice(0, FDP)] if s < BS - 1 else [
                        slice(c * FDC, (c + 1) * FDC) for c in range(NC)]
                    for pi, sl in enumerate(parts):
                        v = nc.vector
                        v.tensor_sub(w5[:, 0:3:2, sl], ms[:, 1:4:2, sl],
                                     ms[:, 2:5:2, sl])
                        v.tensor_add(w5[:, 1:4:2, sl], ms[:, 1:4:2, sl],
                                     ms[:, 2:5:2, sl])
                        v.tensor_scalar_mul(rr[:, 0, sl], w5[:, 2, sl], 2.0)
                        v.tensor_scalar_mul(rr[:, 1, sl], w5[:, 3, sl], 4.0)
                        v.tensor_scalar_mul(w5[:, 4, sl], w5[:, 2, sl], 8.0)
                        v.tensor_add(tu[:, :, sl], w5[:, 1::-1, sl], w5[:, 3:5, sl])
                        v.tensor_add(y[:, 0:4:3, sl], tu[:, :, sl], ms[:, 0:6:5, sl])
                        out_dma(nc, ndma, out_ext[s, ocb, :, 0:4:3, sl],
                                y[:, 0:4:3, sl], last=(s == BS - 1))
                        ndma += 1
                        v.tensor_add(y[:, 1:3, sl], w5[:, 0:2, sl], rr[:, :, sl])
                        out_dma(nc, ndma, out_ext[s, ocb, :, 1:3, sl],
                                y[:, 1:3, sl], last=(s == BS - 1))
                        ndma += 1
                if s + 3 < BS and ocb == 1:
                    fetch(s + 3)
    nc.compile()
    return nc


def _host_prep(x, demog_label, kernel_base, kernel_mask):
    B = x.shape[0]
    # pad h and w by 1 (h to 58, w to 58)
    xpad = np.zeros((B, IC, HP, HP), np.float32)
    xpad[:, :, 1:H + 1, 1:W + 1] = x
    # input transform: V[b, ic, i, h, j] = sum_t BT4[i,t] xpad[b, ic, h, 4j+t]
    dwin = np.lib.stride_tricks.sliding_window_view(xpad, A, axis=3)[:, :, :, ::4, :]
    V = np.einsum("it,bchjt->bcihj", BT4.astype(np.float32), dwin,
                  optimize=True).astype(bfloat16)
    # weight transform: W[b, ic, i*3+kh, oc] = sum_kw G4[i,kw] kb[oc,ic,kh,kw] m[b,ic,kh,kw]
    mg = kernel_mask[demog_label]                        # [B, IC, 3, 3]
    km = np.einsum("ochw,bchw->bochw", kernel_base, mg, optimize=True)
    Wt = np.einsum("iw,bochw->bciho", G4.astype(np.float32),
                   km, optimize=True)                    # [B, IC, A, KH, OC]
    Wt = np.ascontiguousarray(Wt.reshape(B, IC, A * NKH, OC)).astype(bfloat16)
    return V, Wt


def _host_post(buf):
    # buf: [B, 2, 128, 4, 784] bf16 -> [B, 256, 56, 56] f32
    B = buf.shape[0]
    r = buf.reshape(B, 2, 128, 4, H, NT).astype(np.float32)
    r = r.transpose(0, 1, 2, 4, 5, 3)                    # [B, 2, 128, 56, 14, 4]
    return np.ascontiguousarray(r.reshape(B, OC, H, W))


def run(inputs, trace=False, **kw):
    from concourse.bass_utils import run_bass_kernel_spmd

    global _cached_nc
    if _cached_nc is None:
        _cached_nc = _build()
    nc = _cached_nc

    x = np.asarray(inputs["x"], dtype=np.float32)
    demog_label = np.asarray(inputs["demog_label"])
    kernel_base = np.asarray(inputs["kernel_base"], dtype=np.float32)
    kernel_mask = np.asarray(inputs["kernel_mask"], dtype=np.float32)

    V, Wt = _host_prep(x, demog_label, kernel_base, kernel_mask)

    in_maps = []
    for c in range(NCORES):
        sl = slice(c * BS, (c + 1) * BS)
        in_maps.append({
            "v": np.ascontiguousarray(V[sl]),
            "w": np.ascontiguousarray(Wt[sl]),
        })

    last_exc = None
    for _attempt in range(3):
        try:
            res = run_bass_kernel_spmd(nc, in_maps, core_ids=list(range(NCORES)),
                                       trace=trace, **kw)
            outs = [_host_post(np.asarray(r["out"])) for r in res.results]
            full = np.concatenate(outs, axis=0)
            return full, res
        except Exception as e:  # transient NRT/device faults: retry
            last_exc = e
            import time
            time.sleep(10)
    raise last_exc


def kernel(**inputs):
    out, _ = run(inputs, trace=False)
    return out
